# revision 53
# baseline (speedup 1.0000x reference)
"""Trainium2 Bass kernel for nn_Encoder_7413113553686.

Key algebraic fact exploited: the reference loops
    out = x0
    for i in range(L): out = _guidance(x0, q_w[i], kv_w[i], proj_w[i], proj_b[i])
where every iteration consumes the SAME x0 — so the result is just the LAST
block (i = L-1 = 20) applied to x0.  Everything else is dead compute.

Computation per full output:
    patches = im2col(sam)                 # [B, 1024, 64]
    x  = patches @ Wc + conv_b            # conv as GEMM -> [B, 1024, 768]
    x0 = LN(x) * g + b + pos
    q = x0 @ qw ; k,v = x0 @ kvw ; per-head attn softmax(q k^T / sqrt(96)) v
    out = attn_out @ pw + pb + x0

Sharding over 8 cores: core c = (b, g) with b = c>>1 (batch), g = c&1
(head-group: heads 4g..4g+3).  Each core computes x0 for its batch
(duplicated across the pair — tiny), its 4 heads of attention, and a partial
projection (its 384 columns of the head-concat).  Both cores of a pair add
0.5*x0 + pb/2 so the host-side pair-sum reconstructs the full residual+bias.

This version uses mixed fp8e4/bf16/f32r precision (validated ~1.3e-3 rel err
on CPU sim vs the 2e-2 gate):
  - fp8 DoubleRow matmuls (2 K-planes per instruction, ~2.7x f32r throughput
    on HW) for the q/k/v GEMMs, attn@V and the projection.
  - bf16 q/k for the scores GEMM (K=96 is a single chunk, so DoubleRow
    doesn't apply; bf16 streams at the same 220ns/512col as fp8 and avoids
    an extra quantization).
  - f32r for the conv/x0/residual path, which dominates the output norm.
  - LN statistics via a host-precomputed Gram matrix: ss[t] = p^T (G p) with
    G = Wc Wc^T [65x65], so the conv GEMM runs ONCE: the patches are
    pre-scaled by rstd (a [65,N] DVE op) and the -mu*rstd correction rides
    the GEMM as an extra K row.
  - All power-of-2 scale bookkeeping (fp8 dynamic range) folds into the
    PSUM evictions.
The scalar (ACT) engine is reserved for softmax exp during attention — at
~1.1us per [128,1024] tile x 32 tiles it is the critical resource.
"""

import os
import sys

import numpy as np

for _p in ("/opt/trn_rl_repo",):
    if os.path.isdir(_p) and _p not in sys.path:
        sys.path.insert(0, _p)

import ml_dtypes  # noqa: E402

from concourse import bacc, bass, mybir, tile  # noqa: E402
from concourse.bass_utils import run_bass_kernel_spmd  # noqa: E402

F32 = mybir.dt.float32
F32R = mybir.dt.float32r
F8 = mybir.dt.float8e4
BF16 = mybir.dt.bfloat16
DR = mybir.MatmulPerfMode.DoubleRow

B, D, N, NH, HD = 4, 768, 1024, 8, 96
SCALE = float(HD) ** -0.5
LAYER = 20
AF = mybir.ActivationFunctionType

# power-of-2 scale plan (host-folded into weights / on-chip evictions):
#   x8   = fp8(8 * x0)
#   qw8  = fp8(256 * qw * SCALE)   kw8 = fp8(32 * kw)   vw8 = fp8(32 * vw)
#   q_b  = bf16(psum_q / 32) = 64*q      k_b = bf16(psum_k / 32) = 8*k
#   scores_psum = 512 * scores  ->  exp(scale=1/512)
#   v8   = fp8(psum_v / 16) = 16*v ; ones col = 16
#   oT8  = fp8((po * 32) * (1/den))  = 32 * attn_out
#   pw8  = fp8(64 * pw)
#   proj_psum = 2048 * proj ; half-eye = 1024 -> +1024*x0 ; evict * 2^-11


def _body(nc, tc, io, outT):
    mm = nc.tensor.matmul

    import contextlib
    _persist_ctx = contextlib.ExitStack()
    persist = _persist_ctx.enter_context(
        tc.tile_pool(name="persist", bufs=1))

    def ptile(name, shape, dtype=F32):
        return persist.tile(shape, dtype, tag=name, name=name)

    # ---------------- input DMAs, ordered by first use ----------------
    # row 96 of sb_pT is filled at runtime with the LN mean row (32-aligned
    # partition: engines only allow partition remaps at multiples of 32).
    # Rows 65:96 are zeroed; the matching conv weight rows are zero too.
    # One aligned [97,N] multiply by the rstd broadcast then yields the
    # scaled patches AND the mu*rstd correction row for the conv GEMM.
    # LN statistics tolerate fp8 (~0.5% on rstd): an fp8 copy of the patches
    # and Gram matrix goes FIRST on the wire (~112KB) so the stats chain can
    # start ~7us earlier; the bf16 patches (for the conv) stream afterwards.
    ga8 = ptile("ga8", [65, 97], F8)
    nc.sync.dma_start(out=ga8[:, :], in_=io["ga8"][:, :])
    pT8 = ptile("pT8", [65, N], F8)
    nc.sync.dma_start(out=pT8[:, 0:512], in_=io["pT8"][:, 0:512])
    nc.sync.dma_start(out=pT8[:, 512:1024], in_=io["pT8"][:, 512:1024])
    onesc = ptile("onesc", [65, 1], F8)
    nc.sync.dma_start(out=onesc[:, :], in_=io["onesc"][:, :])
    sb_pT = ptile("sb_pT", [97, N], BF16)
    sb_wc2 = ptile("sb_wc2", [97, D], BF16)
    # rows 65:96 arrive zeroed from the host (row 96 is overwritten with the
    # LN mean at runtime).  wc2 is split and interleaved so conv m0 can start
    # as soon as its weight chunk and the first patch half are in.
    nc.sync.dma_start(out=sb_pT[0:96, 0:512], in_=io["pT"][:, 0:512])
    nc.sync.dma_start(out=sb_wc2[:, 0:256], in_=io["wc2"][:, 0:256])
    nc.sync.dma_start(out=sb_pT[0:96, 512:1024], in_=io["pT"][:, 512:1024])
    onesr97 = ptile("onesr97", [1, 97], F32R)
    nc.gpsimd.dma_start(out=onesr97[:, :], in_=io["onesr97"][:, :])
    eps_col = ptile("eps_col", [1, 1])
    nc.gpsimd.memset(eps_col[:, :], 1e-5)
    with tc.tile_pool(name="boot_ps", bufs=1, space="PSUM") as boot_ps:
        boot = boot_ps.tile([1, 1], F32, name="boot")
        nc.tensor.matmul(boot[:, :], eps_col[:, :], eps_col[:, :],
                         start=True, stop=True)
    warm_bc = ptile("warm_bc", [2, 1])
    nc.gpsimd.partition_broadcast(warm_bc[:, :], eps_col[:, :])
    warm_ln = ptile("warm_ln", [1, 1])
    # first Ln use pays a ~1.3us ACT table load; do it here, during the DMA
    # ramp, instead of inside the latency-critical LN-stats chain
    nc.scalar.activation(warm_ln[:, :], eps_col[:, :], AF.Ln)

    # pos rows prefetch right away (needed early in the conv pipeline);
    # the second wc2 chunk (conv m2+) slots between pos1 and pos2
    pos_sb = [ptile(f"pos{m}", [128, N], BF16) for m in range(6)]
    for m in range(6):
        if m == 2:
            nc.sync.dma_start(out=sb_wc2[:, 256:768],
                              in_=io["wc2"][:, 256:768])
        nc.sync.dma_start(out=pos_sb[m][:, :],
                          in_=io["posT"][m, :, :])

    qw8, kw8, vw8 = [], [], []
    for j in range(3):
        for lst, nm, dram in ((qw8, "qw8", io["qw8"]), (kw8, "kw8", io["kw8"]),
                              (vw8, "vw8", io["vw8"])):
            t = ptile(f"{nm}{j}", [128, 2, 384], F8)
            nc.sync.dma_start(out=t[:, :, :], in_=dram[j, :, :, :])
            lst.append(t)
    pw8 = []
    for hp in range(2):
        t = ptile(f"pw8{hp}", [97, 2, D], F8)
        nc.sync.dma_start(out=t[:, :, :], in_=io["pw8"][hp, :, :, :])
        pw8.append(t)

    # persistent activations (oT8 row 96 = ones: carries the pb bias via a
    # matching pw8 row, so no separate bias matmul is needed)
    p2 = ptile("p2", [97, N], BF16)        # rstd-scaled patches + mur row
    x0T = [ptile(f"x0T{m}", [128, N], BF16) for m in range(6)]
    x8 = [ptile(f"x8_{j}", [128, 2, N], F8) for j in range(3)]
    v8 = [ptile(f"v8_{mp}", [128, 2, 4, 128], F8) for mp in range(4)]
    oT8 = [ptile(f"oT8_{hp}", [97, 2, N], F8) for hp in range(2)]
    srow = ptile("srow", [128, N])  # head h uses partition h*32 (32-aligned)
    rstd = ptile("rstd", [1, N], F32R)

    with (
        tc.tile_pool(name="ps", bufs=2, space="PSUM") as ps,
        tc.tile_pool(name="wk", bufs=2) as wk,
        tc.tile_pool(name="expp", bufs=2) as expp,
    ):
        lp = nc.allow_low_precision

        # ---------------- LN statistics via the Gram matrix ----------------
        # pg rows 0:65 = (G p), row 96 = mu (G_aug col 96 carries colsum/D)
        pg = ps.tile([97, N], F32, tag="acc", name="pg")
        for n in range(2):
            sl = bass.ts(n, 512)
            mm(pg[:, sl], ga8[:, :], pT8[:, sl], start=True, stop=True)
        # (Gram n=0 only needs the first patch-half DMA; n=1 the second)
        pgp = wk.tile([65, N], F8, tag="pgp", name="pgp", bufs=1)
        with lp(reason="fp8 stats"):
            for n in range(2):
                sl = bass.ts(n, 512)
                nc.vector.tensor_mul(pgp[:, sl], pT8[:, sl], pg[0:65, sl])
            # mu row -> same partition (96) of the bf16 patches tile
            # (pg row 96 is 128*mu from the fp8 range scaling)
            nc.vector.tensor_scalar_mul(sb_pT[96:97, :], pg[96:97, :],
                                        1.0 / 128.0)
        ps_ss = ps.tile([1, N], F32, tag="big", name="ps_ss")
        for n in range(2):
            sl = bass.ts(n, 512)
            mm(ps_ss[:, sl], onesc[:, :], pgp[:, sl], start=True, stop=True)

        # G'' = G - D c c^T folds the mean-centering into the Gram matrix,
        # so ps_ss is the CENTRAL sum of squares: var = ps_ss / D.
        # rstd = exp(-0.5 * ln(var + eps)) — Ln and Exp share one ACT table set
        lnv = wk.tile([1, N], F32, tag="row", name="lnv")
        # single full-width Ln then Exp: they live in different ACT table
        # sets, and the scheduler would interleave split halves, paying the
        # 1.3us table load three times instead of once
        nc.scalar.activation(lnv[:, :], ps_ss[:, :], AF.Ln,
                             bias=eps_col[:, :], scale=1.0 / (16.0 * D))
        with lp(reason="f32r rstd"):
            nc.scalar.activation(rstd[:, :], lnv[:, :], AF.Exp, scale=-0.5)
        # broadcast rstd over 97 partitions with a K=1 PE matmul (the gpsimd
        # broadcast library takes ~12us to load at first use)
        for n in range(2):
            sl = bass.ts(n, 512)
            rb97 = ps.tile([97, 512], F32, tag="big", name=f"rb97_{n}")
            mm(rb97[:, :], onesr97[:, :], rstd[:, sl], start=True, stop=True)
            with lp(reason="bf16 patches"):
                nc.vector.tensor_mul(p2[:, sl], sb_pT[:, sl], rb97[:, :])

        qb_t, kb_t = [None] * 4, [None] * 4

        pq_t, pk_t = [None] * 4, [None] * 4

        def emit_q_mm(h, j, tag="big"):
            hs = slice(h * 96, (h + 1) * 96)
            if j == 0:
                pq_t[h] = ps.tile([96, N], F32, tag=tag, name=f"pq{h}")
            pq = pq_t[h]
            for n in range(2):
                sl = bass.ts(n, 512)
                mm(pq[:, sl], qw8[j][:, :, hs], x8[j][:, :, sl],
                   start=(j == 0), stop=(j == 2), perf_mode=DR)

        def emit_k_mm(h, j, tag="big"):
            hs = slice(h * 96, (h + 1) * 96)
            if j == 0:
                pk_t[h] = ps.tile([96, N], F32, tag=tag, name=f"pk{h}")
            pk = pk_t[h]
            for n in range(2):
                sl = bass.ts(n, 512)
                mm(pk[:, sl], kw8[j][:, :, hs], x8[j][:, :, sl],
                   start=(j == 0), stop=(j == 2), perf_mode=DR)

        def emit_q_ev(h):
            qb_t[h] = wk.tile([96, N], BF16, tag="qb", name=f"qb{h}", bufs=2)
            with lp(reason="bf16 qk"):
                nc.vector.tensor_scalar_mul(qb_t[h][:, :], pq_t[h][:, :],
                                            1.0 / 32.0)

        def emit_k_ev(h):
            kb_t[h] = wk.tile([96, N], BF16, tag="kb", name=f"kb{h}", bufs=2)
            with lp(reason="bf16 qk"):
                nc.vector.tensor_scalar_mul(kb_t[h][:, :], pk_t[h][:, :],
                                            1.0 / 32.0)

        def emit_q(h, tag="big"):
            for j in range(3):
                emit_q_mm(h, j, tag)
            emit_q_ev(h)

        def emit_k(h, tag="big"):
            for j in range(3):
                emit_k_mm(h, j, tag)
            emit_k_ev(h)


        # ---------------- conv (single pass) + pos -> x0, x8 ----------------
        for m in range(6):
            pc = ps.tile([128, N], F32, tag="big", name=f"pc{m}")
            for n in range(2):
                sl = bass.ts(n, 512)
                mm(pc[:, sl], sb_wc2[:, m * 128:(m + 1) * 128], p2[:, sl],
                   start=True, stop=True)
                # x0T holds 0.5*x0 (the residual each core contributes):
                # wc2 and pos are pre-halved on the host; x8 rescales by 16
                with lp(reason="x0 bf16"):
                    nc.vector.tensor_add(x0T[m][:, sl], pc[:, sl],
                                         pos_sb[m][:, sl])
                with lp(reason="x8 fp8"):
                    nc.scalar.activation(x8[m // 2][:, m % 2, sl],
                                         x0T[m][:, sl], AF.Copy, scale=16.0)
            # head-0 q/k accumulate j-chunks (acc pool) as x8 planes appear,
            # hiding them entirely inside the conv window
            if m in (1, 3, 5):
                emit_q_mm(0, m // 2, tag="acc")
            if m in (2, 4):
                emit_k_mm(0, m // 2 - 1, tag="acc")
        emit_k_mm(0, 2, tag="acc")
        emit_q_ev(0)
        emit_k_ev(0)

        # ---------------- V = x8 @ vw8 (token-major, fp8 DR) ----------------
        for mp in range(4):
            nc.gpsimd.memset(v8[mp][:, :, :, 96:128], 0.5)
        for hp in range(2):
            nc.gpsimd.memset(oT8[hp][96:97, :, :], 1.0)

        # ---------------- per-head attention ----------------
        def emit_norm(h):
            # The 1-lane reciprocal of a [1, 1024] row costs 6.6us on DVE;
            # instead spread the row over all 128 partitions with a reshape
            # DMA, reciprocal at full width (~0.2us), and DMA back.
            dt_r = F32R if h == 3 else F32
            s_pk = wk.tile([128, 8], F32, tag="spk", name=f"spk{h}", bufs=1)
            nc.sync.dma_start(out=s_pk[:, :], in_=srow[h * 32:h * 32 + 1, :])
            r_pk = wk.tile([128, 8], dt_r, tag="rpk", name=f"rpk{h}", bufs=1)
            with lp(reason="softmax denom reciprocal"):
                # the v8 ones column is 0.5 (vs the 16 of the values), so
                # 1/den is directly 32/(16*sum ex): the fp8 oT scale for free
                nc.vector.reciprocal(r_pk[:, :], s_pk[:, :])
            recip = wk.tile([1, N], dt_r, tag="row2", name=f"rc{h}", bufs=1)
            nc.sync.dma_start(out=recip[:, :], in_=r_pk[:, :])
            po = po_t[h]
            if h < 3:
                # rb in SBUF via gpsimd (the oT8 mul reads po from PSUM and
                # DVE tensor ops cannot read two PSUM operands)
                rb = wk.tile([96, N], F32, tag="rb", name=f"rb{h}", bufs=1)
                nc.gpsimd.partition_broadcast(rb[:, :], recip[:, :])
                with lp(reason="oT8 fp8"):
                    nc.vector.tensor_mul(oT8[h // 2][0:96, h % 2, :],
                                         po[0:96, :], rb[:, :])
            else:
                # critical tail: evict po to SBUF on ACT (idle) in parallel
                # with the reciprocal round-trip, broadcast on the PE, and
                # multiply SBUF x PSUM
                po3 = wk.tile([96, N], F32, tag="po3", name="po3", bufs=1)
                nc.scalar.copy(po3[:, :], po[0:96, :])
                for n in range(2):
                    sl = bass.ts(n, 512)
                    rbp = ps.tile([96, 512], F32, tag="big", name=f"rbp{n}")
                    mm(rbp[:, :], onesr97[:, 0:96], recip[:, sl],
                       start=True, stop=True)
                    with lp(reason="oT8 fp8"):
                        nc.vector.tensor_mul(oT8[h // 2][0:96, h % 2, sl],
                                             po3[:, sl], rbp[:, :])

        po_t = [None] * 4
        for hh in range(4):
            emit_q(hh)
            emit_k(hh)
        for m in range(8):
            pv = ps.tile([128, 384], F32, tag="acc", name=f"pv{m}")
            for j in range(3):
                mm(pv[:, :], x8[j][:, :, m * 128:(m + 1) * 128], vw8[j][:, :, :],
                   start=(j == 0), stop=(j == 2), perf_mode=DR)
            with lp(reason="v8 fp8"):
                nc.vector.tensor_scalar_mul(
                    v8[m // 2][:, m % 2, :, 0:96],
                    pv.rearrange("p (h d) -> p h d", h=4), 1.0 / 16.0)
        for h in range(4):
            qb, kb = qb_t[h], kb_t[h]
            po = ps.tile([98, N], F32, tag="acc", name=f"po{h}")
            po_t[h] = po
            for m in range(8):
                pss = ps.tile([128, N], F32, tag="big", name=f"pss{h}_{m}")
                for n in range(2):
                    sl = bass.ts(n, 512)
                    mm(pss[:, sl], kb[:, m * 128:(m + 1) * 128], qb[:, sl],
                       start=True, stop=True)
                ext = expp.tile([128, 2, N], F8, tag="exp", name=f"ex{h}_{m}") \
                    if m % 2 == 0 else ext
                with lp(reason="exp fp8"):
                    nc.scalar.activation(ext[:, m % 2, :], pss[:, :], AF.Exp,
                                         scale=1.0 / 512.0)
                if m % 2 == 1:
                    mp = m // 2
                    for n in range(2):
                        sl = bass.ts(n, 512)
                        mm(po[:, sl], v8[mp][:, :, h, 0:98], ext[:, :, sl],
                           start=(mp == 0), stop=(mp == 3), perf_mode=DR)
                if m == 2 and h < 3:
                    emit_q(h + 1)  # next head's q, overlapped
                if m == 5 and h < 3:
                    emit_k(h + 1)  # next head's k, overlapped
                if m == 0 and h >= 1:
                    emit_norm(h - 1)  # previous head's normalize, overlapped
                if m == 6 and h == 3:
                    # pre-start proj m0 pair0 (oT8 pair 0 has long been ready)
                    pp0 = ps.tile([128, N], F32, tag="acc", name="pp0")
                    for n2 in range(2):
                        sl2 = bass.ts(n2, 512)
                        mm(pp0[:, sl2], pw8[0][:, :, 0:128], oT8[0][:, :, sl2],
                           start=True, stop=False, perf_mode=DR)
            # denominator row (feeds the reciprocal)
            nc.vector.tensor_copy(srow[h * 32:h * 32 + 1, :], po[96:97, :])
        emit_norm(3)

        # ---------------- proj tail: all-DR + fused scale-add residual ----------------
        # pair-0 groups (ready as soon as heads 0/1 are normalized) run one
        # step ahead of the norm(3)-gated pair-1 groups, so the PE never
        # idles during the norm(3) latency and PSUM stays within 2 buffers.
        pp_t = {0: pp0}

        def emit_pair0(m):
            msl = slice(m * 128, (m + 1) * 128)
            pp = ps.tile([128, N], F32, tag="big", name=f"pp{m}")
            pp_t[m] = pp
            for n in range(2):
                sl = bass.ts(n, 512)
                mm(pp[:, sl], pw8[0][:, :, msl], oT8[0][:, :, sl],
                   start=True, stop=False, perf_mode=DR)

        def emit_pair1(m):
            msl = slice(m * 128, (m + 1) * 128)
            pp = pp_t[m]
            for n in range(2):
                sl = bass.ts(n, 512)
                mm(pp[:, sl], pw8[1][:, :, msl], oT8[1][:, :, sl],
                   start=False, stop=(n == 1), perf_mode=DR)
            ou = wk.tile([128, N], BF16, tag="out", name=f"ou{m}")
            with lp(reason="bf16 out"):
                nc.vector.scalar_tensor_tensor(ou[:, :], pp[:, :], 2.0 ** -11,
                                               x0T[m][:, :],
                                               mybir.AluOpType.mult,
                                               mybir.AluOpType.add)
            for n in range(2):
                sl = bass.ts(n, 512)
                eng = nc.sync if (2 * m + n) % 2 == 0 else nc.scalar
                eng.dma_start(out=outT[m * 128:(m + 1) * 128, sl],
                              in_=ou[:, sl])

        emit_pair0(1)
        emit_pair0(2)
        for m in range(6):
            emit_pair1(m)
            if m + 3 <= 5:
                emit_pair0(m + 3)


def _build_nc():
    nc = bacc.Bacc("TRN2", target_bir_lowering=False, debug=False,
                   enable_asserts=False)
    io = {}
    for name, shape, dt in (
        ("pT", [96, N], BF16), ("ga8", [65, 97], F8),
        ("pT8", [65, N], F8),
        ("onesc", [65, 1], F8), ("wc2", [97, D], BF16),
        ("onesr97", [1, 97], F32R),
        ("posT", [6, 128, N], BF16),
        ("qw8", [3, 128, 2, 384], F8), ("kw8", [3, 128, 2, 384], F8),
        ("vw8", [3, 128, 2, 384], F8), ("pw8", [2, 97, 2, D], F8),
    ):
        io[name] = nc.dram_tensor(name, shape, dt, kind="ExternalInput").ap()
    outT = nc.dram_tensor("outT", [D, N], BF16, kind="ExternalOutput").ap()
    with tile.TileContext(nc) as tc:
        _body(nc, tc, io, outT)
    nc.compile()
    return nc


_NC_CACHE = {}


def _get_nc():
    if "nc" not in _NC_CACHE:
        _NC_CACHE["nc"] = _build_nc()
    return _NC_CACHE["nc"]


def _prep_in_maps(sam, conv_w, conv_b, ln_g, ln_b, pos, q_w, kv_w, proj_w,
                  proj_b):
    f = np.float32
    f8 = ml_dtypes.float8_e4m3
    bf = ml_dtypes.bfloat16
    sam = np.asarray(sam, f)
    qwL = (np.asarray(q_w[LAYER], f) * SCALE).astype(f)
    kvL = np.asarray(kv_w[LAYER], f)
    kwL, vwL = kvL[:, :D], kvL[:, D:]
    pwL = np.ascontiguousarray(np.asarray(proj_w[LAYER], f))
    pbL = np.asarray(proj_b[LAYER], f)
    g = np.asarray(ln_g, f)

    # Wc [65, 768] = [patch weights ; conv bias row], UNSCALED by gamma —
    # used for the LN statistics (Gram) and, gamma-scaled, for the conv.
    W2 = np.asarray(conv_w, f).reshape(D, 64).T            # [64, 768]
    Wc = np.concatenate([W2, np.asarray(conv_b, f)[None, :]], 0)  # [65, 768]
    G = Wc @ Wc.T                                          # [65, 65]
    # mu column/row lives at index 96 (32-aligned partition); 65:96 zero
    c = Wc.sum(1) / D
    ga = np.zeros((65, 97), f)
    # fp8 range: G'' entries ~0.3, c ~5e-3 -> scale G by 4 and c by 128; the
    # Gram output is then 4*Gp (harmless: pgp and ss scale by 4, and the Ln
    # scale folds the 4 away) and mu arrives as 128*mu (folded below).
    ga[:, 0:65] = (G - D * np.outer(c, c)) * 16.0
    ga[:, 96] = c * 128.0
    ga8 = np.ascontiguousarray(ga).astype(f8)
    wc2 = np.zeros((97, D), f)
    wc2[0:65] = Wc * g[None, :]
    wc2[96] = -g
    wc2 = (wc2 * 0.5).astype(bf)

    posT_eff = (0.5 * (np.asarray(ln_b, f)[:, None]
                + np.asarray(pos, f).T)).reshape(6, 128, N).astype(bf)


    def pack_k2(w, s):
        # [768, 384] -> [3, 128, 2, 384] fp8 with K-plane pairs
        return np.ascontiguousarray(
            (w * s).reshape(3, 2, 128, 384).transpose(0, 2, 1, 3)).astype(f8)

    in_maps = []
    for c in range(8):
        b, gg = c >> 1, c & 1
        img = sam[b, 0]
        patches = img.reshape(32, 8, 32, 8).transpose(0, 2, 1, 3).reshape(1024, 64)
        pT_aug = np.zeros((96, N), bf)
        pT_aug[0:64] = patches.T
        pT_aug[64] = 1.0
        pT8 = np.zeros((65, N), np.float32)
        pT8[0:64] = patches.T
        pT8[64] = 1.0
        pT8 = pT8.astype(f8)
        sl = slice(gg * 384, (gg + 1) * 384)
        pw8 = np.zeros((2, 97, 2, D), np.float32)
        pw8[:, 0:96] = (pwL[sl, :] * 64.0).reshape(2, 2, 96, D).transpose(0, 2, 1, 3)
        # the oT8 ones row (partition 96) carries pb/2 * 2048 on (hp0, plane0)
        pw8[0, 96, 0, :] = pbL * 1024.0
        pw8 = np.ascontiguousarray(pw8).astype(f8)
        in_maps.append({
            "pT": pT_aug,
            "pT8": pT8,
            "ga8": ga8,
            "onesc": np.ones((65, 1), f8),
            "onesr97": np.ones((1, 97), f),
            "wc2": wc2,
            "posT": posT_eff,
            "qw8": pack_k2(qwL[:, sl], 256.0),
            "kw8": pack_k2(kwL[:, sl], 32.0),
            "vw8": pack_k2(vwL[:, sl], 32.0),
            "pw8": pw8,
        })
    return in_maps


def kernel(sam, conv_w, conv_b, ln_g, ln_b, pos, q_w, kv_w, proj_w, proj_b,
           **_unused):
    nc = _get_nc()
    in_maps = _prep_in_maps(sam, conv_w, conv_b, ln_g, ln_b, pos, q_w, kv_w,
                            proj_w, proj_b)
    res = run_bass_kernel_spmd(nc, in_maps, core_ids=list(range(8)))
    outs = [np.asarray(r["outT"], dtype=np.float32) for r in res.results]
    full = np.stack([(outs[2 * b] + outs[2 * b + 1]).T for b in range(B)])
    return np.ascontiguousarray(full.astype(np.float32))


if __name__ == "__main__":
    # quick smoke test against the reference when run in the problem dir
    sys.path.insert(0, os.path.dirname(os.path.abspath(__file__)))
    import reference as R

    inputs = {k: np.asarray(v) for k, v in R.setup_inputs().items()}
    expected = np.asarray(R.reference(**inputs))
    actual = kernel(**inputs)
    rel = np.linalg.norm(actual - expected) / np.linalg.norm(expected)
    print("Relative error:", rel)


# revision 55
# speedup vs baseline: 1.0961x; 1.0961x over previous
"""Trainium2 Bass kernel for nn_Encoder_7413113553686.

Key algebraic fact exploited: the reference loops
    out = x0
    for i in range(L): out = _guidance(x0, q_w[i], kv_w[i], proj_w[i], proj_b[i])
where every iteration consumes the SAME x0 — so the result is just the LAST
block (i = L-1 = 20) applied to x0.  Everything else is dead compute.

Computation per full output:
    patches = im2col(sam)                 # [B, 1024, 64]
    x  = patches @ Wc + conv_b            # conv as GEMM -> [B, 1024, 768]
    x0 = LN(x) * g + b + pos
    q = x0 @ qw ; k,v = x0 @ kvw ; per-head attn softmax(q k^T / sqrt(96)) v
    out = attn_out @ pw + pb + x0

Sharding over 8 cores: core c = (b, g) with b = c>>1 (batch), g = c&1
(head-group: heads 4g..4g+3).  Each core computes x0 for its batch
(duplicated across the pair — tiny), its 4 heads of attention, and a partial
projection (its 384 columns of the head-concat).  Both cores of a pair add
0.5*x0 + pb/2 so the host-side pair-sum reconstructs the full residual+bias.

This version uses mixed fp8e4/bf16/f32r precision (validated ~1.3e-3 rel err
on CPU sim vs the 2e-2 gate):
  - fp8 DoubleRow matmuls (2 K-planes per instruction, ~2.7x f32r throughput
    on HW) for the q/k/v GEMMs, attn@V and the projection.
  - bf16 q/k for the scores GEMM (K=96 is a single chunk, so DoubleRow
    doesn't apply; bf16 streams at the same 220ns/512col as fp8 and avoids
    an extra quantization).
  - f32r for the conv/x0/residual path, which dominates the output norm.
  - LN statistics via a host-precomputed Gram matrix: ss[t] = p^T (G p) with
    G = Wc Wc^T [65x65], so the conv GEMM runs ONCE: the patches are
    pre-scaled by rstd (a [65,N] DVE op) and the -mu*rstd correction rides
    the GEMM as an extra K row.
  - All power-of-2 scale bookkeeping (fp8 dynamic range) folds into the
    PSUM evictions.
The scalar (ACT) engine is reserved for softmax exp during attention — at
~1.1us per [128,1024] tile x 32 tiles it is the critical resource.
"""

import os
import sys

import numpy as np

for _p in ("/opt/trn_rl_repo",):
    if os.path.isdir(_p) and _p not in sys.path:
        sys.path.insert(0, _p)

import ml_dtypes  # noqa: E402

from concourse import bacc, bass, mybir, tile  # noqa: E402
from concourse.bass_utils import run_bass_kernel_spmd  # noqa: E402

F32 = mybir.dt.float32
F32R = mybir.dt.float32r
F8 = mybir.dt.float8e4
BF16 = mybir.dt.bfloat16
DR = mybir.MatmulPerfMode.DoubleRow

B, D, N, NH, HD = 4, 768, 1024, 8, 96
SCALE = float(HD) ** -0.5
LAYER = 20
AF = mybir.ActivationFunctionType

# power-of-2 scale plan (host-folded into weights / on-chip evictions):
#   x8   = fp8(8 * x0)
#   qw8  = fp8(256 * qw * SCALE)   kw8 = fp8(32 * kw)   vw8 = fp8(32 * vw)
#   q_b  = bf16(psum_q / 32) = 64*q      k_b = bf16(psum_k / 32) = 8*k
#   scores_psum = 512 * scores  ->  exp(scale=1/512)
#   v8   = fp8(psum_v / 16) = 16*v ; ones col = 16
#   oT8  = fp8((po * 32) * (1/den))  = 32 * attn_out
#   pw8  = fp8(64 * pw)
#   proj_psum = 2048 * proj ; half-eye = 1024 -> +1024*x0 ; evict * 2^-11


def _body(nc, tc, io, outT):
    mm = nc.tensor.matmul

    import contextlib
    _persist_ctx = contextlib.ExitStack()
    persist = _persist_ctx.enter_context(
        tc.tile_pool(name="persist", bufs=1))

    def ptile(name, shape, dtype=F32):
        return persist.tile(shape, dtype, tag=name, name=name)

    # ---------------- input DMAs, ordered by first use ----------------
    # row 96 of sb_pT is filled at runtime with the LN mean row (32-aligned
    # partition: engines only allow partition remaps at multiples of 32).
    # Rows 65:96 are zeroed; the matching conv weight rows are zero too.
    # One aligned [97,N] multiply by the rstd broadcast then yields the
    # scaled patches AND the mu*rstd correction row for the conv GEMM.
    # LN statistics tolerate fp8 (~0.5% on rstd): an fp8 copy of the patches
    # and Gram matrix goes FIRST on the wire (~112KB) so the stats chain can
    # start ~7us earlier; the bf16 patches (for the conv) stream afterwards.
    ga8 = ptile("ga8", [65, 97], F8)
    nc.sync.dma_start(out=ga8[:, :], in_=io["ga8"][:, :])
    pT8 = ptile("pT8", [65, N], F8)
    nc.sync.dma_start(out=pT8[:, 0:512], in_=io["pT8"][:, 0:512])
    nc.sync.dma_start(out=pT8[:, 512:1024], in_=io["pT8"][:, 512:1024])
    onesc = ptile("onesc", [65, 1], F8)
    nc.sync.dma_start(out=onesc[:, :], in_=io["onesc"][:, :])
    sb_pT = ptile("sb_pT", [97, N], BF16)
    sb_wc2 = ptile("sb_wc2", [97, D], BF16)
    # rows 65:96 arrive zeroed from the host (row 96 is overwritten with the
    # LN mean at runtime).  wc2 is split and interleaved so conv m0 can start
    # as soon as its weight chunk and the first patch half are in.
    nc.sync.dma_start(out=sb_pT[0:96, 0:512], in_=io["pT"][:, 0:512])
    nc.sync.dma_start(out=sb_wc2[:, 0:256], in_=io["wc2"][:, 0:256])
    nc.sync.dma_start(out=sb_pT[0:96, 512:1024], in_=io["pT"][:, 512:1024])
    onesr97 = ptile("onesr97", [1, 97], F32R)
    nc.gpsimd.dma_start(out=onesr97[:, :], in_=io["onesr97"][:, :])
    eps_col = ptile("eps_col", [1, 1])
    nc.gpsimd.memset(eps_col[:, :], 1e-5)
    with tc.tile_pool(name="boot_ps", bufs=1, space="PSUM") as boot_ps:
        boot = boot_ps.tile([1, 1], F32, name="boot")
        nc.tensor.matmul(boot[:, :], eps_col[:, :], eps_col[:, :],
                         start=True, stop=True)
    warm_bc = ptile("warm_bc", [2, 1])
    nc.gpsimd.partition_broadcast(warm_bc[:, :], eps_col[:, :])
    warm_ln = ptile("warm_ln", [1, 1])
    # first Ln use pays a ~1.3us ACT table load; do it here, during the DMA
    # ramp, instead of inside the latency-critical LN-stats chain
    nc.scalar.activation(warm_ln[:, :], eps_col[:, :], AF.Ln)

    # pos rows prefetch right away (needed early in the conv pipeline);
    # the second wc2 chunk (conv m2+) slots between pos1 and pos2
    pos_sb = [ptile(f"pos{m}", [128, N], BF16) for m in range(6)]
    for m in range(6):
        if m == 2:
            nc.sync.dma_start(out=sb_wc2[:, 256:768],
                              in_=io["wc2"][:, 256:768])
        nc.sync.dma_start(out=pos_sb[m][:, :],
                          in_=io["posT"][m, :, :])

    qw8, kw8, vw8 = [], [], []
    for j in range(3):
        for lst, nm, dram in ((qw8, "qw8", io["qw8"]), (kw8, "kw8", io["kw8"]),
                              (vw8, "vw8", io["vw8"])):
            t = ptile(f"{nm}{j}", [128, 2, 384], F8)
            nc.sync.dma_start(out=t[:, :, :], in_=dram[j, :, :, :])
            lst.append(t)
    pw8 = []
    for hp in range(2):
        t = ptile(f"pw8{hp}", [97, 2, D], F8)
        nc.sync.dma_start(out=t[:, :, :], in_=io["pw8"][hp, :, :, :])
        pw8.append(t)

    # persistent activations (oT8 row 96 = ones: carries the pb bias via a
    # matching pw8 row, so no separate bias matmul is needed)
    p2 = ptile("p2", [97, N], BF16)        # rstd-scaled patches + mur row
    x0T = [ptile(f"x0T{m}", [128, N], BF16) for m in range(6)]
    x8 = [ptile(f"x8_{j}", [128, 2, N], F8) for j in range(3)]
    v8 = [ptile(f"v8_{mp}", [128, 2, 4, 128], F8) for mp in range(4)]
    oT8 = [ptile(f"oT8_{hp}", [97, 2, N], F8) for hp in range(2)]
    srow = ptile("srow", [128, N])  # head h uses partition h*32 (32-aligned)
    rstd = ptile("rstd", [1, N], F32R)

    with (
        tc.tile_pool(name="ps", bufs=2, space="PSUM") as ps,
        tc.tile_pool(name="wk", bufs=2) as wk,
        tc.tile_pool(name="expp", bufs=2) as expp,
    ):
        lp = nc.allow_low_precision

        # ---------------- LN statistics via the Gram matrix ----------------
        # pg rows 0:65 = (G p), row 96 = mu (G_aug col 96 carries colsum/D)
        pg = ps.tile([97, N], F32, tag="acc", name="pg")
        for n in range(2):
            sl = bass.ts(n, 512)
            mm(pg[:, sl], ga8[:, :], pT8[:, sl], start=True, stop=True)
        # (Gram n=0 only needs the first patch-half DMA; n=1 the second)
        pgp = wk.tile([65, N], F8, tag="pgp", name="pgp", bufs=1)
        with lp(reason="fp8 stats"):
            for n in range(2):
                sl = bass.ts(n, 512)
                nc.vector.tensor_mul(pgp[:, sl], pT8[:, sl], pg[0:65, sl])
            # mu row -> same partition (96) of the bf16 patches tile
            # (pg row 96 is 128*mu from the fp8 range scaling)
            nc.vector.tensor_scalar_mul(sb_pT[96:97, :], pg[96:97, :],
                                        1.0 / 128.0)
        ps_ss = ps.tile([1, N], F32, tag="big", name="ps_ss")
        for n in range(2):
            sl = bass.ts(n, 512)
            mm(ps_ss[:, sl], onesc[:, :], pgp[:, sl], start=True, stop=True)

        # G'' = G - D c c^T folds the mean-centering into the Gram matrix,
        # so ps_ss is the CENTRAL sum of squares: var = ps_ss / D.
        # rstd = exp(-0.5 * ln(var + eps)) — Ln and Exp share one ACT table set
        lnv = wk.tile([1, N], F32, tag="row", name="lnv")
        # single full-width Ln then Exp: they live in different ACT table
        # sets, and the scheduler would interleave split halves, paying the
        # 1.3us table load three times instead of once
        nc.scalar.activation(lnv[:, :], ps_ss[:, :], AF.Ln,
                             bias=eps_col[:, :], scale=1.0 / (16.0 * D))
        with lp(reason="f32r rstd"):
            nc.scalar.activation(rstd[:, :], lnv[:, :], AF.Exp, scale=-0.5)
        # broadcast rstd over 97 partitions with a K=1 PE matmul (the gpsimd
        # broadcast library takes ~12us to load at first use)
        for n in range(2):
            sl = bass.ts(n, 512)
            rb97 = ps.tile([97, 512], F32, tag="big", name=f"rb97_{n}")
            mm(rb97[:, :], onesr97[:, :], rstd[:, sl], start=True, stop=True)
            with lp(reason="bf16 patches"):
                nc.vector.tensor_mul(p2[:, sl], sb_pT[:, sl], rb97[:, :])

        qb_t, kb_t = [None] * 4, [None] * 4

        pq_t, pk_t = [None] * 4, [None] * 4

        def emit_q_mm(h, j, tag="big"):
            hs = slice(h * 96, (h + 1) * 96)
            if j == 0:
                pq_t[h] = ps.tile([96, N], F32, tag=tag, name=f"pq{h}")
            pq = pq_t[h]
            for n in range(2):
                sl = bass.ts(n, 512)
                mm(pq[:, sl], qw8[j][:, :, hs], x8[j][:, :, sl],
                   start=(j == 0), stop=(j == 2), perf_mode=DR)

        def emit_k_mm(h, j, tag="big"):
            hs = slice(h * 96, (h + 1) * 96)
            if j == 0:
                pk_t[h] = ps.tile([96, N], F32, tag=tag, name=f"pk{h}")
            pk = pk_t[h]
            for n in range(2):
                sl = bass.ts(n, 512)
                mm(pk[:, sl], kw8[j][:, :, hs], x8[j][:, :, sl],
                   start=(j == 0), stop=(j == 2), perf_mode=DR)

        def emit_q_ev(h):
            qb_t[h] = wk.tile([96, N], BF16, tag="qb", name=f"qb{h}", bufs=4)
            with lp(reason="bf16 qk"):
                nc.vector.tensor_scalar_mul(qb_t[h][:, :], pq_t[h][:, :],
                                            1.0 / 32.0)

        def emit_k_ev(h):
            kb_t[h] = wk.tile([96, N], BF16, tag="kb", name=f"kb{h}", bufs=4)
            with lp(reason="bf16 qk"):
                nc.vector.tensor_scalar_mul(kb_t[h][:, :], pk_t[h][:, :],
                                            1.0 / 32.0)

        def emit_q(h, tag="big"):
            for j in range(3):
                emit_q_mm(h, j, tag)
            emit_q_ev(h)

        def emit_k(h, tag="big"):
            for j in range(3):
                emit_k_mm(h, j, tag)
            emit_k_ev(h)


        # ---------------- conv (single pass) + pos -> x0, x8 ----------------
        for m in range(6):
            pc = ps.tile([128, N], F32, tag="big", name=f"pc{m}")
            for n in range(2):
                sl = bass.ts(n, 512)
                mm(pc[:, sl], sb_wc2[:, m * 128:(m + 1) * 128], p2[:, sl],
                   start=True, stop=True)
                # x0T holds 0.5*x0 (the residual each core contributes):
                # wc2 and pos are pre-halved on the host; x8 rescales by 16
                with lp(reason="x0 bf16"):
                    nc.vector.tensor_add(x0T[m][:, sl], pc[:, sl],
                                         pos_sb[m][:, sl])
                with lp(reason="x8 fp8"):
                    nc.scalar.activation(x8[m // 2][:, m % 2, sl],
                                         x0T[m][:, sl], AF.Copy, scale=16.0)
            # head-0 q/k accumulate j-chunks (acc pool) as x8 planes appear,
            # hiding them entirely inside the conv window
            if m in (1, 3, 5):
                emit_q_mm(0, m // 2, tag="acc")
            if m in (2, 4):
                emit_k_mm(0, m // 2 - 1, tag="acc")
        emit_k_mm(0, 2, tag="acc")
        emit_q_ev(0)
        emit_k_ev(0)

        # ---------------- V = x8 @ vw8 (token-major, fp8 DR) ----------------
        for mp in range(4):
            nc.gpsimd.memset(v8[mp][:, :, :, 96:128], 0.5)
        for hp in range(2):
            nc.gpsimd.memset(oT8[hp][96:97, :, :], 1.0)

        # ---------------- per-head attention ----------------
        def emit_norm(h):
            # The 1-lane reciprocal of a [1, 1024] row costs 6.6us on DVE;
            # instead spread the row over all 128 partitions with a reshape
            # DMA, reciprocal at full width (~0.2us), and DMA back.
            dt_r = F32R if h == 3 else F32
            s_pk = wk.tile([128, 8], F32, tag="spk", name=f"spk{h}", bufs=1)
            nc.sync.dma_start(out=s_pk[:, :], in_=srow[h * 32:h * 32 + 1, :])
            r_pk = wk.tile([128, 8], dt_r, tag="rpk", name=f"rpk{h}", bufs=1)
            with lp(reason="softmax denom reciprocal"):
                # the v8 ones column is 0.5 (vs the 16 of the values), so
                # 1/den is directly 32/(16*sum ex): the fp8 oT scale for free
                nc.vector.reciprocal(r_pk[:, :], s_pk[:, :])
            recip = wk.tile([1, N], dt_r, tag="row2", name=f"rc{h}", bufs=1)
            nc.sync.dma_start(out=recip[:, :], in_=r_pk[:, :])
            po = po_t[h]
            if h < 3:
                # rb in SBUF via gpsimd (the oT8 mul reads po from PSUM and
                # DVE tensor ops cannot read two PSUM operands)
                rb = wk.tile([96, N], F32, tag="rb", name=f"rb{h}", bufs=1)
                nc.gpsimd.partition_broadcast(rb[:, :], recip[:, :])
                with lp(reason="oT8 fp8"):
                    nc.vector.tensor_mul(oT8[h // 2][0:96, h % 2, :],
                                         po[0:96, :], rb[:, :])
            else:
                # critical tail: evict po to SBUF on ACT (idle) in parallel
                # with the reciprocal round-trip, broadcast on the PE, and
                # multiply SBUF x PSUM
                po3 = wk.tile([96, N], F32, tag="po3", name="po3", bufs=1)
                nc.scalar.copy(po3[:, :], po[0:96, :])
                for n in range(2):
                    sl = bass.ts(n, 512)
                    rbp = ps.tile([96, 512], F32, tag="big", name=f"rbp{n}")
                    mm(rbp[:, :], onesr97[:, 0:96], recip[:, sl],
                       start=True, stop=True)
                    with lp(reason="oT8 fp8"):
                        nc.vector.tensor_mul(oT8[h // 2][0:96, h % 2, sl],
                                             po3[:, sl], rbp[:, :])

        po_t = [None] * 4
        for hh in range(4):
            emit_q(hh)
            emit_k(hh)
        for m in range(8):
            pv = ps.tile([128, 384], F32, tag="acc", name=f"pv{m}")
            for j in range(3):
                mm(pv[:, :], x8[j][:, :, m * 128:(m + 1) * 128], vw8[j][:, :, :],
                   start=(j == 0), stop=(j == 2), perf_mode=DR)
            with lp(reason="v8 fp8"):
                nc.vector.tensor_scalar_mul(
                    v8[m // 2][:, m % 2, :, 0:96],
                    pv.rearrange("p (h d) -> p h d", h=4), 1.0 / 16.0)
        for h in range(4):
            qb, kb = qb_t[h], kb_t[h]
            po = ps.tile([98, N], F32, tag="acc", name=f"po{h}")
            po_t[h] = po
            for m in range(8):
                pss = ps.tile([128, N], F32, tag="big", name=f"pss{h}_{m}")
                for n in range(2):
                    sl = bass.ts(n, 512)
                    mm(pss[:, sl], kb[:, m * 128:(m + 1) * 128], qb[:, sl],
                       start=True, stop=True)
                ext = expp.tile([128, 2, N], F8, tag="exp", name=f"ex{h}_{m}") \
                    if m % 2 == 0 else ext
                with lp(reason="exp fp8"):
                    nc.scalar.activation(ext[:, m % 2, :], pss[:, :], AF.Exp,
                                         scale=1.0 / 512.0)
                if m % 2 == 1:
                    mp = m // 2
                    for n in range(2):
                        sl = bass.ts(n, 512)
                        mm(po[:, sl], v8[mp][:, :, h, 0:98], ext[:, :, sl],
                           start=(mp == 0), stop=(mp == 3), perf_mode=DR)
                if m == 0 and h >= 1:
                    emit_norm(h - 1)  # previous head's normalize, overlapped
                if m == 6 and h == 3:
                    # pre-start proj m0 pair0 (oT8 pair 0 has long been ready)
                    pp0 = ps.tile([128, N], F32, tag="acc", name="pp0")
                    for n2 in range(2):
                        sl2 = bass.ts(n2, 512)
                        mm(pp0[:, sl2], pw8[0][:, :, 0:128], oT8[0][:, :, sl2],
                           start=True, stop=False, perf_mode=DR)
            # denominator row (feeds the reciprocal)
            nc.vector.tensor_copy(srow[h * 32:h * 32 + 1, :], po[96:97, :])
        emit_norm(3)

        # ---------------- proj tail: all-DR + fused scale-add residual ----------------
        # pair-0 groups (ready as soon as heads 0/1 are normalized) run one
        # step ahead of the norm(3)-gated pair-1 groups, so the PE never
        # idles during the norm(3) latency and PSUM stays within 2 buffers.
        pp_t = {0: pp0}

        def emit_pair0(m):
            msl = slice(m * 128, (m + 1) * 128)
            pp = ps.tile([128, N], F32, tag="big", name=f"pp{m}")
            pp_t[m] = pp
            for n in range(2):
                sl = bass.ts(n, 512)
                mm(pp[:, sl], pw8[0][:, :, msl], oT8[0][:, :, sl],
                   start=True, stop=False, perf_mode=DR)

        def emit_pair1(m):
            msl = slice(m * 128, (m + 1) * 128)
            pp = pp_t[m]
            for n in range(2):
                sl = bass.ts(n, 512)
                mm(pp[:, sl], pw8[1][:, :, msl], oT8[1][:, :, sl],
                   start=False, stop=(n == 1), perf_mode=DR)
            ou = wk.tile([128, N], BF16, tag="out", name=f"ou{m}")
            with lp(reason="bf16 out"):
                nc.vector.scalar_tensor_tensor(ou[:, :], pp[:, :], 2.0 ** -11,
                                               x0T[m][:, :],
                                               mybir.AluOpType.mult,
                                               mybir.AluOpType.add)
            for n in range(2):
                sl = bass.ts(n, 512)
                eng = nc.sync if (2 * m + n) % 2 == 0 else nc.scalar
                eng.dma_start(out=outT[m * 128:(m + 1) * 128, sl],
                              in_=ou[:, sl])

        emit_pair0(1)
        emit_pair0(2)
        for m in range(6):
            emit_pair1(m)
            if m + 3 <= 5:
                emit_pair0(m + 3)


def _build_nc():
    nc = bacc.Bacc("TRN2", target_bir_lowering=False, debug=False,
                   enable_asserts=False)
    io = {}
    for name, shape, dt in (
        ("pT", [96, N], BF16), ("ga8", [65, 97], F8),
        ("pT8", [65, N], F8),
        ("onesc", [65, 1], F8), ("wc2", [97, D], BF16),
        ("onesr97", [1, 97], F32R),
        ("posT", [6, 128, N], BF16),
        ("qw8", [3, 128, 2, 384], F8), ("kw8", [3, 128, 2, 384], F8),
        ("vw8", [3, 128, 2, 384], F8), ("pw8", [2, 97, 2, D], F8),
    ):
        io[name] = nc.dram_tensor(name, shape, dt, kind="ExternalInput").ap()
    outT = nc.dram_tensor("outT", [D, N], BF16, kind="ExternalOutput").ap()
    with tile.TileContext(nc) as tc:
        _body(nc, tc, io, outT)
    nc.compile()
    return nc


_NC_CACHE = {}


def _get_nc():
    if "nc" not in _NC_CACHE:
        _NC_CACHE["nc"] = _build_nc()
    return _NC_CACHE["nc"]


def _prep_in_maps(sam, conv_w, conv_b, ln_g, ln_b, pos, q_w, kv_w, proj_w,
                  proj_b):
    f = np.float32
    f8 = ml_dtypes.float8_e4m3
    bf = ml_dtypes.bfloat16
    sam = np.asarray(sam, f)
    qwL = (np.asarray(q_w[LAYER], f) * SCALE).astype(f)
    kvL = np.asarray(kv_w[LAYER], f)
    kwL, vwL = kvL[:, :D], kvL[:, D:]
    pwL = np.ascontiguousarray(np.asarray(proj_w[LAYER], f))
    pbL = np.asarray(proj_b[LAYER], f)
    g = np.asarray(ln_g, f)

    # Wc [65, 768] = [patch weights ; conv bias row], UNSCALED by gamma —
    # used for the LN statistics (Gram) and, gamma-scaled, for the conv.
    W2 = np.asarray(conv_w, f).reshape(D, 64).T            # [64, 768]
    Wc = np.concatenate([W2, np.asarray(conv_b, f)[None, :]], 0)  # [65, 768]
    G = Wc @ Wc.T                                          # [65, 65]
    # mu column/row lives at index 96 (32-aligned partition); 65:96 zero
    c = Wc.sum(1) / D
    ga = np.zeros((65, 97), f)
    # fp8 range: G'' entries ~0.3, c ~5e-3 -> scale G by 4 and c by 128; the
    # Gram output is then 4*Gp (harmless: pgp and ss scale by 4, and the Ln
    # scale folds the 4 away) and mu arrives as 128*mu (folded below).
    ga[:, 0:65] = (G - D * np.outer(c, c)) * 16.0
    ga[:, 96] = c * 128.0
    ga8 = np.ascontiguousarray(ga).astype(f8)
    wc2 = np.zeros((97, D), f)
    wc2[0:65] = Wc * g[None, :]
    wc2[96] = -g
    wc2 = (wc2 * 0.5).astype(bf)

    posT_eff = (0.5 * (np.asarray(ln_b, f)[:, None]
                + np.asarray(pos, f).T)).reshape(6, 128, N).astype(bf)


    def pack_k2(w, s):
        # [768, 384] -> [3, 128, 2, 384] fp8 with K-plane pairs
        return np.ascontiguousarray(
            (w * s).reshape(3, 2, 128, 384).transpose(0, 2, 1, 3)).astype(f8)

    in_maps = []
    for c in range(8):
        b, gg = c >> 1, c & 1
        img = sam[b, 0]
        patches = img.reshape(32, 8, 32, 8).transpose(0, 2, 1, 3).reshape(1024, 64)
        pT_aug = np.zeros((96, N), bf)
        pT_aug[0:64] = patches.T
        pT_aug[64] = 1.0
        pT8 = np.zeros((65, N), np.float32)
        pT8[0:64] = patches.T
        pT8[64] = 1.0
        pT8 = pT8.astype(f8)
        sl = slice(gg * 384, (gg + 1) * 384)
        pw8 = np.zeros((2, 97, 2, D), np.float32)
        pw8[:, 0:96] = (pwL[sl, :] * 64.0).reshape(2, 2, 96, D).transpose(0, 2, 1, 3)
        # the oT8 ones row (partition 96) carries pb/2 * 2048 on (hp0, plane0)
        pw8[0, 96, 0, :] = pbL * 1024.0
        pw8 = np.ascontiguousarray(pw8).astype(f8)
        in_maps.append({
            "pT": pT_aug,
            "pT8": pT8,
            "ga8": ga8,
            "onesc": np.ones((65, 1), f8),
            "onesr97": np.ones((1, 97), f),
            "wc2": wc2,
            "posT": posT_eff,
            "qw8": pack_k2(qwL[:, sl], 256.0),
            "kw8": pack_k2(kwL[:, sl], 32.0),
            "vw8": pack_k2(vwL[:, sl], 32.0),
            "pw8": pw8,
        })
    return in_maps


def kernel(sam, conv_w, conv_b, ln_g, ln_b, pos, q_w, kv_w, proj_w, proj_b,
           **_unused):
    nc = _get_nc()
    in_maps = _prep_in_maps(sam, conv_w, conv_b, ln_g, ln_b, pos, q_w, kv_w,
                            proj_w, proj_b)
    res = run_bass_kernel_spmd(nc, in_maps, core_ids=list(range(8)))
    outs = [np.asarray(r["outT"], dtype=np.float32) for r in res.results]
    full = np.stack([(outs[2 * b] + outs[2 * b + 1]).T for b in range(B)])
    return np.ascontiguousarray(full.astype(np.float32))


if __name__ == "__main__":
    # quick smoke test against the reference when run in the problem dir
    sys.path.insert(0, os.path.dirname(os.path.abspath(__file__)))
    import reference as R

    inputs = {k: np.asarray(v) for k, v in R.setup_inputs().items()}
    expected = np.asarray(R.reference(**inputs))
    actual = kernel(**inputs)
    rel = np.linalg.norm(actual - expected) / np.linalg.norm(expected)
    print("Relative error:", rel)
